# revision 16
# baseline (speedup 1.0000x reference)
"""Trainium2 Bass kernel for nn_Attention2d (sparse_attention) — v2.

Math (per reference):
  x: (2, 128, 64, 64); T = 4096 tokens; 4 heads x 32 channels.
  qkv 1x1-conv -> per-head attention over T -> 1x1-conv out proj -> residual.

Sharding: one (batch, head) pair per core (8 cores). Each core computes its
head's attention fully on-chip and returns the per-head partial of the
output projection (bf16); the host sums the 4 head partials per batch and
adds the residual + biases (exact, fp32).

v2 changes vs baseline:
  - exp is SPLIT between ScalarE (exact activation-Exp, cols [0:ES)) and
    VectorE (Schraudolph exp2 bit-trick: one tensor_scalar mult+add writing
    int16 bit patterns that ARE the bf16 of 2^(y/128), cols [ES:2048)).
    Both paths carry the same global 2^((C2-16256)/128) factor, which
    cancels in the softmax ratio. ES=1152 keeps QK chunks 0,1 dependent
    only on ScalarE and chunk 3 only on VectorE, decoupling the pipelines.
  - PV matmuls are 2-way column-tiled by t-half (out is only 64 partitions:
    32 v channels + 32 ones rows for the denominator): t-cols [0:256) of a
    t-block accumulate into PSUM partitions 0:64, t-cols [256:512) into
    64:128, concurrently. No cross-half merge is needed.
  - epilogue: VectorE fast-reciprocal of the denominators, ScalarE copies
    the numerators out of PSUM, GpSimd does the normalize multiply,
    ScalarE copies the projected output; output DMA'd as bf16 (host
    accumulates in fp32).
"""

import numpy as np
import ml_dtypes

B, C, Hh, Ww = 2, 128, 64, 64
T = Hh * Ww          # 4096
NH, CH = 4, 32
SCALE2 = float(1.0 / np.sqrt(CH))
N_CORES = 8
NSUP = T // 512      # 8 supers per t-block, 8 t-blocks

# Schraudolph exp2 constants (VectorE path): bits = round(raw*C1 + C2),
# bits viewed as bf16 == 2^((bits-16256)/128) ~= exp(raw*SCALE2) * 2^((C2-16256)/128)
C1 = SCALE2 * float(np.log2(np.e)) * 128.0
C2 = 16256.0 - 7.4
# ScalarE path matches the global factor so both halves share one scale:
SBIAS = float((C2 - 16256.0) / 128.0 * np.log(2.0))

ES = 1280            # ScalarE exp columns per 2048-col super; VectorE rest

_cache = {}


def _build_nc(debug=False):
    import concourse.tile as tile
    from concourse import bacc, mybir

    BF16 = mybir.dt.bfloat16
    F32 = mybir.dt.float32
    I16 = mybir.dt.int16
    Exp = mybir.ActivationFunctionType.Exp
    MULT = mybir.AluOpType.mult
    ADD = mybir.AluOpType.add

    nc = bacc.Bacc("TRN2", target_bir_lowering=False, debug=False,
                   num_devices=N_CORES)
    dbg = {}
    if debug:
        dbg["q"] = nc.dram_tensor("dq", [128, T], BF16, kind="ExternalOutput")
        dbg["k"] = nc.dram_tensor("dk", [128, T], BF16, kind="ExternalOutput")
        dbg["vT"] = nc.dram_tensor("dvT", [128, 2048], BF16,
                                   kind="ExternalOutput")
        dbg["st"] = nc.dram_tensor("dst", [128, 2048], F32,
                                   kind="ExternalOutput")
        dbg["p"] = nc.dram_tensor("dp", [128, 2048], BF16,
                                  kind="ExternalOutput")
        dbg["pv"] = nc.dram_tensor("dpv", [128, 512], F32,
                                   kind="ExternalOutput")
        dbg["num"] = nc.dram_tensor("dnum", [32, 512], F32,
                                    kind="ExternalOutput")
        dbg["rc"] = nc.dram_tensor("drc", [32, 512], F32,
                                   kind="ExternalOutput")
        dbg["an"] = nc.dram_tensor("dan", [32, T], BF16,
                                   kind="ExternalOutput")

    x_in = nc.dram_tensor("x", [128, T], BF16, kind="ExternalInput")
    wq_in = nc.dram_tensor("wqT", [128, 128], BF16, kind="ExternalInput")
    wk_in = nc.dram_tensor("wkT", [128, 128], BF16, kind="ExternalInput")
    wv_in = nc.dram_tensor("wvT", [128, 32], BF16, kind="ExternalInput")
    wp_in = nc.dram_tensor("wpT", [32, 128], BF16, kind="ExternalInput")
    bq_in = nc.dram_tensor("bq", [128, 1], F32, kind="ExternalInput")
    bk_in = nc.dram_tensor("bk", [128, 1], F32, kind="ExternalInput")
    out_t = nc.dram_tensor("out", [128, T], BF16, kind="ExternalOutput")

    with tile.TileContext(nc) as tc:
        with (
            tc.tile_pool(name="const", bufs=1) as cpool,
            tc.tile_pool(name="work", bufs=2) as wpool,
            tc.tile_pool(name="psum", bufs=1, space="PSUM") as pspool,
        ):
            x_sb = cpool.tile([128, T], BF16)
            for c in range(4):
                nc.sync.dma_start(x_sb[:, c * 1024:(c + 1) * 1024],
                                  x_in[:, c * 1024:(c + 1) * 1024])
            wq_sb = cpool.tile([128, 128], BF16)
            nc.sync.dma_start(wq_sb[:], wq_in[:])
            wk_sb = cpool.tile([128, 128], BF16)
            nc.sync.dma_start(wk_sb[:], wk_in[:])
            wv_sb = cpool.tile([128, 32], BF16)
            nc.sync.dma_start(wv_sb[:], wv_in[:])
            wp_sb = cpool.tile([32, 128], BF16)
            nc.sync.dma_start(wp_sb[:], wp_in[:])
            bq_sb = cpool.tile([128, 1], F32)
            nc.sync.dma_start(bq_sb[:], bq_in[:])
            bk_sb = cpool.tile([128, 1], F32)
            nc.sync.dma_start(bk_sb[:], bk_in[:])

            q_sb = cpool.tile([128, T], BF16)
            k_sb = cpool.tile([128, T], BF16)
            vT_sb = cpool.tile([128, 64 * (T // 128)], BF16)  # (128, 2048)
            an_sb = cpool.tile([32, T], BF16)

            nc.gpsimd.memset(vT_sb[:], 1.0)
            sbias_sb = cpool.tile([128, 1], F32)
            nc.gpsimd.memset(sbias_sb[:], SBIAS)

            # 1-bank fp32 ping-pong tiles for projections / out-proj
            def scratch(nm):
                return pspool.tile([128, 512], F32, tag="pp", bufs=2, name=nm)

            def emit_proj(wsb, bsb, dst, c, nm):
                ps = scratch(nm)
                nc.tensor.matmul(ps[:], wsb[:],
                                 x_sb[:, c * 512:(c + 1) * 512],
                                 start=True, stop=True)
                nc.vector.tensor_scalar_add(
                    dst[:, c * 512:(c + 1) * 512], ps[:], bsb[:])

            def emit_vt(half):
                ps = scratch(f"pp_v{half}")
                for j16 in range(16):
                    j = half * 16 + j16
                    nc.tensor.matmul(
                        ps[:, j16 * 32:(j16 + 1) * 32],
                        x_sb[:, j * 128:(j + 1) * 128],
                        wv_sb[:],
                        start=True, stop=True)
                src = ps[:].rearrange("p (j c) -> p j c", c=32)
                dstv = vT_sb[:].rearrange("p (j c) -> p j c", c=64)
                nc.vector.tensor_copy(
                    dstv[:, half * 16:(half + 1) * 16, 0:32], src)

            # ---- prologue: only what super 0 needs ----
            emit_proj(wk_sb, bk_sb, k_sb, 0, "pp_k0")
            emit_proj(wq_sb, bq_sb, q_sb, 0, "pp_q0")

            # ---- attention, software-pipelined over 64 supers ----
            state = {}
            pv_tiles = {}

            def emit_score_exp(jg):
                tb, j = divmod(jg, NSUP)
                if j == 0:
                    pv_tiles[tb] = pspool.tile(
                        [128, 512], F32, tag="pv", bufs=2, name=f"pv_{tb}")
                st = pspool.tile([128, 2048], F32, tag="st", bufs=1,
                                 name=f"st_{jg}")
                tsl = slice(tb * 512, (tb + 1) * 512)
                for g in range(4):
                    sblk = 4 * j + g
                    nc.tensor.matmul(
                        st[:, g * 512:(g + 1) * 512],
                        k_sb[32 * g:32 * (g + 1), 128 * sblk:128 * (sblk + 1)],
                        q_sb[32 * g:32 * (g + 1), tsl],
                        start=True, stop=True,
                        tile_position=(32 * g, 0))
                p_sb = wpool.tile([128, 2048], BF16, tag="p", bufs=3)
                # exp split: ScalarE takes [0:ES), VectorE bit-trick the rest
                nc.scalar.activation(p_sb[:, 0:ES], st[:, 0:ES], Exp,
                                     bias=sbias_sb[:], scale=SCALE2)
                nc.vector.tensor_scalar(
                    p_sb[:, ES:2048].bitcast(I16), st[:, ES:2048],
                    C1, C2, MULT, ADD)
                if debug and jg == 0:
                    dst_sb = wpool.tile([128, 2048], F32, tag="dbgst")
                    nc.vector.tensor_copy(dst_sb[:], st[:])
                    nc.sync.dma_start(dbg["st"][:], dst_sb[:])
                    nc.sync.dma_start(dbg["p"][:], p_sb[:])
                state[jg] = p_sb

            def emit_pv(jg):
                tb, j = divmod(jg, NSUP)
                p_sb = state.pop(jg)
                pv = pv_tiles[tb]
                for g in range(4):
                    sblk = 4 * j + g
                    # start=True clears has_written for the written
                    # partition range (x the bank's columns); the two
                    # column-tile halves write disjoint partitions, so each
                    # half's first matmul clears independently.
                    for h in range(2):
                        nc.tensor.matmul(
                            pv[64 * h:64 * h + 64,
                               256 * h:256 * (h + 1)],
                            vT_sb[:, 64 * sblk:64 * (sblk + 1)],
                            p_sb[:, g * 512 + 256 * h:g * 512 + 256 * (h + 1)],
                            start=(j == 0 and g == 0),
                            stop=(j == NSUP - 1 and g == 3),
                            tile_position=(0, 64 * h),
                            skip_group_check=True)
                if j == NSUP - 1:
                    # t-block epilogue per t-half: reciprocal of denominator
                    # rows (VectorE), numerator copy out of PSUM (ScalarE),
                    # normalize (GpSimd), project (PE), store copy (ScalarE)
                    tsl = slice(tb * 512, (tb + 1) * 512)
                    if debug and tb == 0:
                        dpv_sb = wpool.tile([128, 512], F32, tag="dbgpv")
                        nc.vector.tensor_copy(dpv_sb[:], pv[:])
                        nc.sync.dma_start(dbg["pv"][:], dpv_sb[:])
                    for h in range(2):
                        tq = slice(tb * 512 + 256 * h,
                                   tb * 512 + 256 * (h + 1))
                        csl = slice(256 * h, 256 * (h + 1))
                        a_h = wpool.tile([64, 256], F32, tag=f"ah{h}")
                        nc.scalar.copy(a_h[:], pv[64 * h:64 * h + 64, csl])
                        dcp = wpool.tile([32, 256], F32, tag=f"dcp{h}")
                        nc.vector.tensor_copy(dcp[:], a_h[32:64, :])
                        rc = wpool.tile([32, 256], F32, tag=f"rc{h}")
                        nc.vector.reciprocal_approx_fast(rc[:], dcp[:])
                        nc.vector.tensor_mul(an_sb[:, tq], a_h[0:32, :],
                                             rc[:])
                        if debug and tb == 0:
                            nc.sync.dma_start(
                                dbg["num"][:, 256 * h:256 * (h + 1)],
                                a_h[0:32, :])
                            nc.sync.dma_start(
                                dbg["rc"][:, 256 * h:256 * (h + 1)], rc[:])
                    op = scratch(f"pp_o{tb}")
                    nc.tensor.matmul(op[:], wp_sb[:], an_sb[:, tsl],
                                     start=True, stop=True)
                    o_sb = wpool.tile([128, 512], BF16, tag="o")
                    nc.scalar.copy(o_sb[:], op[:])
                    nc.sync.dma_start(out_t[:, tsl], o_sb[:])

            for jg in range(NSUP * NSUP):
                emit_score_exp(jg)
                # stagger remaining input prep into the supers that have
                # slack, one chunk ahead of the super that needs it
                if jg == 0:
                    emit_vt(0)
                if jg == 1:
                    emit_vt(1)
                if 0 <= jg <= 6:
                    emit_proj(wk_sb, bk_sb, k_sb, jg + 1, f"pp_k{jg + 1}")
                if jg % NSUP == 4 and jg < 56:
                    c = jg // NSUP + 1
                    emit_proj(wq_sb, bq_sb, q_sb, c, f"pp_q{c}")
                if jg >= 1:
                    emit_pv(jg - 1)
            emit_pv(NSUP * NSUP - 1)
            if debug:
                nc.sync.dma_start(dbg["q"][:], q_sb[:])
                nc.sync.dma_start(dbg["k"][:], k_sb[:])
                nc.sync.dma_start(dbg["vT"][:], vT_sb[:])
                nc.sync.dma_start(dbg["an"][:], an_sb[:])

    nc.compile()
    return nc


def _get_nc(debug=False):
    key = ("nc", debug)
    if key not in _cache:
        _cache[key] = _build_nc(debug)
    return _cache[key]


def _make_in_maps(x_, w_qkv, b_qkv, w_proj):
    bf16 = ml_dtypes.bfloat16
    in_maps = []
    for core in range(N_CORES):
        b, g = divmod(core, NH)
        wq = w_qkv[96 * g:96 * g + 32]
        wk = w_qkv[96 * g + 32:96 * g + 64]
        wv = w_qkv[96 * g + 64:96 * g + 96]
        in_maps.append({
            "x": x_[b].astype(bf16),
            "wqT": np.ascontiguousarray(np.tile(wq, (4, 1)).T).astype(bf16),
            "wkT": np.ascontiguousarray(np.tile(wk, (4, 1)).T).astype(bf16),
            "wvT": np.ascontiguousarray(wv.T).astype(bf16),
            "wpT": np.ascontiguousarray(
                w_proj[:, 32 * g:32 * (g + 1)].T).astype(bf16),
            "bq": np.ascontiguousarray(
                np.tile(b_qkv[96 * g:96 * g + 32], 4).reshape(128, 1)),
            "bk": np.ascontiguousarray(
                np.tile(b_qkv[96 * g + 32:96 * g + 64], 4).reshape(128, 1)),
        })
    return in_maps


def _run(x, w_qkv, b_qkv, w_proj, b_proj, trace=False):
    from concourse.bass_utils import run_bass_kernel_spmd

    nc = _get_nc()
    x_ = np.ascontiguousarray(np.asarray(x, np.float32).reshape(B, C, T))
    w_qkv = np.asarray(w_qkv, np.float32)
    b_qkv = np.asarray(b_qkv, np.float32)
    w_proj = np.asarray(w_proj, np.float32)
    b_proj = np.asarray(b_proj, np.float32)

    in_maps = _make_in_maps(x_, w_qkv, b_qkv, w_proj)
    res = run_bass_kernel_spmd(nc, in_maps, core_ids=list(range(N_CORES)),
                               trace=trace)
    out = np.empty((B, C, T), np.float32)
    for b in range(B):
        acc = x_[b] + b_proj[:, None]
        for g in range(NH):
            wp = w_proj[:, 32 * g:32 * (g + 1)]
            bv = b_qkv[96 * g + 64:96 * g + 96]
            acc = acc + res.results[NH * b + g]["out"].astype(np.float32) \
                + (wp @ bv)[:, None]
        out[b] = acc
    return out.reshape(B, C, Hh, Ww), res


def kernel(x, w_qkv, b_qkv, w_proj, b_proj):
    out, _ = _run(x, w_qkv, b_qkv, w_proj, b_proj, trace=False)
    return out.astype(np.asarray(x).dtype)


# revision 17
# speedup vs baseline: 1.2684x; 1.2684x over previous
"""Trainium2 Bass kernel for nn_Attention2d (sparse_attention) — v3.

Math (per reference):
  x: (2, 128, 64, 64); T = 4096 tokens; 4 heads x 32 channels.
  qkv 1x1-conv -> per-head attention over T -> 1x1-conv out proj -> residual.

Sharding: one (batch, head) pair per core (8 cores). Each core computes its
head's attention fully on-chip and returns the per-head partial of the
output projection (bf16); the host sums the 4 head partials per batch and
adds the residual + biases (exact, fp32).

Structure (v3):
  - Scores are produced in supers of 3 s-chunks (1536 fp32 cols = 3 PSUM
    banks), with the st tile DOUBLE-buffered (6 banks): the PE runs QK for
    super n+1 while ScalarE/VectorE exp super n — no serial QK<->exp chain.
  - exp is SPLIT: ScalarE does activation-Exp on cols [0:ES); VectorE does
    a Schraudolph exp2 bit-trick (one tensor_scalar mult+add writing int16
    bit patterns that ARE the bf16 of 2^(y/128)) on the rest. Both carry
    the same global 2^((C2-16256)/128) factor, which cancels in softmax.
  - PV: one matmul per chunk into a [64,512] fp32 accumulator (32 v rows +
    32 ones rows giving the denominator), 32 accumulations per t-block.
  - epilogue per t-block: ScalarE copies the accumulator out of PSUM,
    GpSimd copies the denominator rows, VectorE fast-reciprocal +
    normalize-mul, PE out-projection, ScalarE output copy, bf16 DMA out
    (host accumulates partials in fp32).
  - all inputs arrive in 3 packed DMAs (x / weights / biases).
"""

import numpy as np
import ml_dtypes

B, C, Hh, Ww = 2, 128, 64, 64
T = Hh * Ww          # 4096
NH, CH = 4, 32
SCALE2 = float(1.0 / np.sqrt(CH))
N_CORES = 8
NCH = 32             # s-chunks (of 128 tokens) per t-block

# Schraudolph exp2 constants (VectorE path): bits = round(raw*C1 + C2),
# bits viewed as bf16 == 2^((bits-16256)/128) ~= exp(raw*SCALE2) * 2^((C2-16256)/128)
C1 = SCALE2 * float(np.log2(np.e)) * 128.0
C2 = 16256.0 - 7.4
# ScalarE path matches the global factor so both halves share one scale:
SBIAS = float((C2 - 16256.0) / 128.0 * np.log(2.0))

# super layout within a t-block: 10x 3-chunk + 1x 2-chunk
SUPERS = [(c, min(3, NCH - c)) for c in range(0, NCH, 3)]
ES3, ES2 = 896, 592  # ScalarE exp cols for 1536/1024-col supers

_cache = {}


def _build_nc(debug=False):
    import concourse.tile as tile
    from concourse import bacc, mybir

    BF16 = mybir.dt.bfloat16
    F32 = mybir.dt.float32
    I16 = mybir.dt.int16
    Exp = mybir.ActivationFunctionType.Exp
    MULT = mybir.AluOpType.mult
    ADD = mybir.AluOpType.add

    nc = bacc.Bacc("TRN2", target_bir_lowering=False, debug=False,
                   num_devices=N_CORES)
    dbg = {}
    if debug:
        dbg["q"] = nc.dram_tensor("dq", [128, T], BF16, kind="ExternalOutput")
        dbg["k"] = nc.dram_tensor("dk", [128, T], BF16, kind="ExternalOutput")
        dbg["vT"] = nc.dram_tensor("dvT", [128, 2048], BF16,
                                   kind="ExternalOutput")
        dbg["pv"] = nc.dram_tensor("dpv", [64, 512], F32,
                                   kind="ExternalOutput")
        dbg["rc"] = nc.dram_tensor("drc", [32, 512], F32,
                                   kind="ExternalOutput")
        dbg["an"] = nc.dram_tensor("dan", [32, T], BF16,
                                   kind="ExternalOutput")

    x_in = nc.dram_tensor("x", [128, T], BF16, kind="ExternalInput")
    # packed weights: wqT | wkT | wvT | wpT(padded to 128 rows)
    w_in = nc.dram_tensor("wpack", [128, 416], BF16, kind="ExternalInput")
    b_in = nc.dram_tensor("bpack", [128, 2], F32, kind="ExternalInput")
    out_t = nc.dram_tensor("out", [128, T], BF16, kind="ExternalOutput")

    with tile.TileContext(nc) as tc:
        with (
            tc.tile_pool(name="const", bufs=1) as cpool,
            tc.tile_pool(name="work", bufs=2) as wpool,
            tc.tile_pool(name="psum", bufs=1, space="PSUM") as pspool,
        ):
            x_sb = cpool.tile([128, T], BF16)
            nc.sync.dma_start(x_sb[:], x_in[:])
            w_sb = cpool.tile([128, 416], BF16)
            nc.sync.dma_start(w_sb[:], w_in[:])
            b_sb = cpool.tile([128, 2], F32)
            nc.sync.dma_start(b_sb[:], b_in[:])
            wq_sb = w_sb[:, 0:128]
            wk_sb = w_sb[:, 128:256]
            wv_sb = w_sb[:, 256:288]
            wp_sb = w_sb[0:32, 288:416]
            bq_sb = b_sb[:, 0:1]
            bk_sb = b_sb[:, 1:2]

            q_sb = cpool.tile([128, T], BF16)
            k_sb = cpool.tile([128, T], BF16)
            vT_sb = cpool.tile([128, 64 * NCH], BF16)  # (128, 2048)
            an_sb = cpool.tile([32, T], BF16)

            nc.gpsimd.memset(vT_sb[:], 1.0)
            sbias_sb = cpool.tile([128, 1], F32)
            nc.gpsimd.memset(sbias_sb[:], SBIAS)

            # 1-bank fp32 scratch for projections / vT / out-proj
            def scratch(nm):
                return pspool.tile([128, 512], F32, tag="pp", bufs=1, name=nm)

            def emit_proj(wsb, bsb, dst, c, nm):
                ps = scratch(nm)
                nc.tensor.matmul(ps[:], wsb,
                                 x_sb[:, c * 512:(c + 1) * 512],
                                 start=True, stop=True)
                nc.vector.tensor_scalar_add(
                    dst[:, c * 512:(c + 1) * 512], ps[:], bsb)

            def emit_vt(half):
                ps = scratch(f"pp_v{half}")
                for j16 in range(16):
                    j = half * 16 + j16
                    nc.tensor.matmul(
                        ps[:, j16 * 32:(j16 + 1) * 32],
                        x_sb[:, j * 128:(j + 1) * 128],
                        wv_sb,
                        start=True, stop=True)
                src = ps[:].rearrange("p (j c) -> p j c", c=32)
                dstv = vT_sb[:].rearrange("p (j c) -> p j c", c=64)
                nc.vector.tensor_copy(
                    dstv[:, half * 16:(half + 1) * 16, 0:32], src)

            # ---- prologue: only what super 0 needs ----
            emit_proj(wk_sb, bk_sb, k_sb, 0, "pp_k0")
            emit_proj(wq_sb, bq_sb, q_sb, 0, "pp_q0")

            # ---- attention, software-pipelined over supers ----
            state = {}
            pv_tiles = {}
            supers_all = [(tb, c0, nch) for tb in range(8)
                          for (c0, nch) in SUPERS]

            def emit_score_exp(idx):
                tb, c0, nch = supers_all[idx]
                ncols = nch * 512
                es = ES3 if nch == 3 else ES2
                st = pspool.tile([128, 1536], F32, tag="st", bufs=2,
                                 name=f"st_{idx}")
                tsl = slice(tb * 512, (tb + 1) * 512)
                for ci in range(nch):
                    ch = c0 + ci
                    sblk = tb * 0 + ch  # s-chunk index (global over s)
                    nc.tensor.matmul(
                        st[:, ci * 512:(ci + 1) * 512],
                        k_sb[32 * (ch % 4):32 * (ch % 4) + 32,
                             128 * ch:128 * (ch + 1)],
                        q_sb[32 * (ch % 4):32 * (ch % 4) + 32, tsl],
                        start=True, stop=True,
                        tile_position=(32 * (ch % 4), 0))
                p_sb = wpool.tile([128, 1536], BF16, tag="p", bufs=3)
                nc.scalar.activation(p_sb[:, 0:es], st[:, 0:es], Exp,
                                     bias=sbias_sb[:], scale=SCALE2)
                nc.vector.tensor_scalar(
                    p_sb[:, es:ncols].bitcast(I16), st[:, es:ncols],
                    C1, C2, MULT, ADD)
                state[idx] = p_sb

            def emit_pv(idx):
                tb, c0, nch = supers_all[idx]
                p_sb = state.pop(idx)
                if c0 == 0:
                    pv_tiles[tb] = pspool.tile(
                        [64, 512], F32, tag="pv", bufs=1, name=f"pv_{tb}")
                pv = pv_tiles[tb]
                for ci in range(nch):
                    ch = c0 + ci
                    nc.tensor.matmul(
                        pv[:],
                        vT_sb[:, 64 * ch:64 * (ch + 1)],
                        p_sb[:, ci * 512:(ci + 1) * 512],
                        start=(ch == 0), stop=(ch == NCH - 1),
                        skip_group_check=True)
                if c0 + nch == NCH:
                    # t-block epilogue
                    tsl = slice(tb * 512, (tb + 1) * 512)
                    a_h = wpool.tile([64, 512], F32, tag="ah")
                    nc.scalar.copy(a_h[:], pv[:])
                    dcp = wpool.tile([32, 512], F32, tag="dcp")
                    nc.gpsimd.tensor_copy(dcp[:], a_h[32:64, :])
                    rc = wpool.tile([32, 512], F32, tag="rc")
                    nc.vector.reciprocal_approx_fast(rc[:], dcp[:])
                    nc.vector.tensor_mul(an_sb[:, tsl], a_h[0:32, :], rc[:])
                    if debug and tb == 0:
                        dpv_sb = wpool.tile([64, 512], F32, tag="dbgpv")
                        nc.vector.tensor_copy(dpv_sb[:], pv[:])
                        nc.sync.dma_start(dbg["pv"][:], dpv_sb[:])
                        nc.sync.dma_start(dbg["rc"][:], rc[:])
                    op = scratch(f"pp_o{tb}")
                    nc.tensor.matmul(op[:], wp_sb, an_sb[:, tsl],
                                     start=True, stop=True)
                    o_sb = wpool.tile([128, 512], BF16, tag="o")
                    nc.scalar.copy(o_sb[:], op[:])
                    nc.sync.dma_start(out_t[:, tsl], o_sb[:])

            for idx in range(len(supers_all)):
                emit_score_exp(idx)
                # stagger input prep into the supers that have slack,
                # ahead of the super that needs it
                if idx == 0:
                    emit_vt(0)
                if idx == 1:
                    emit_vt(1)
                if 0 <= idx <= 6:
                    emit_proj(wk_sb, bk_sb, k_sb, idx + 1, f"pp_k{idx + 1}")
                if idx % 11 == 5 and idx < 77:
                    c = idx // 11 + 1
                    emit_proj(wq_sb, bq_sb, q_sb, c, f"pp_q{c}")
                if idx >= 1:
                    emit_pv(idx - 1)
            emit_pv(len(supers_all) - 1)
            if debug:
                nc.sync.dma_start(dbg["q"][:], q_sb[:])
                nc.sync.dma_start(dbg["k"][:], k_sb[:])
                nc.sync.dma_start(dbg["vT"][:], vT_sb[:])
                nc.sync.dma_start(dbg["an"][:], an_sb[:])

    nc.compile()
    return nc


def _get_nc(debug=False):
    key = ("nc", debug)
    if key not in _cache:
        _cache[key] = _build_nc(debug)
    return _cache[key]


def _make_in_maps(x_, w_qkv, b_qkv, w_proj):
    bf16 = ml_dtypes.bfloat16
    in_maps = []
    for core in range(N_CORES):
        b, g = divmod(core, NH)
        wq = w_qkv[96 * g:96 * g + 32]
        wk = w_qkv[96 * g + 32:96 * g + 64]
        wv = w_qkv[96 * g + 64:96 * g + 96]
        wpack = np.zeros((128, 416), np.float32)
        wpack[:, 0:128] = np.tile(wq, (4, 1)).T
        wpack[:, 128:256] = np.tile(wk, (4, 1)).T
        wpack[:, 256:288] = wv.T
        wpack[0:32, 288:416] = w_proj[:, 32 * g:32 * (g + 1)].T
        bpack = np.stack([np.tile(b_qkv[96 * g:96 * g + 32], 4),
                          np.tile(b_qkv[96 * g + 32:96 * g + 64], 4)],
                         axis=1)
        in_maps.append({
            "x": x_[b].astype(bf16),
            "wpack": np.ascontiguousarray(wpack).astype(bf16),
            "bpack": np.ascontiguousarray(bpack.astype(np.float32)),
        })
    return in_maps


def _run(x, w_qkv, b_qkv, w_proj, b_proj, trace=False):
    from concourse.bass_utils import run_bass_kernel_spmd

    nc = _get_nc()
    x_ = np.ascontiguousarray(np.asarray(x, np.float32).reshape(B, C, T))
    w_qkv = np.asarray(w_qkv, np.float32)
    b_qkv = np.asarray(b_qkv, np.float32)
    w_proj = np.asarray(w_proj, np.float32)
    b_proj = np.asarray(b_proj, np.float32)

    in_maps = _make_in_maps(x_, w_qkv, b_qkv, w_proj)
    res = run_bass_kernel_spmd(nc, in_maps, core_ids=list(range(N_CORES)),
                               trace=trace)
    out = np.empty((B, C, T), np.float32)
    for b in range(B):
        acc = x_[b] + b_proj[:, None]
        for g in range(NH):
            wp = w_proj[:, 32 * g:32 * (g + 1)]
            bv = b_qkv[96 * g + 64:96 * g + 96]
            acc = acc + res.results[NH * b + g]["out"].astype(np.float32) \
                + (wp @ bv)[:, None]
        out[b] = acc
    return out.reshape(B, C, Hh, Ww), res


def kernel(x, w_qkv, b_qkv, w_proj, b_proj):
    out, _ = _run(x, w_qkv, b_qkv, w_proj, b_proj, trace=False)
    return out.astype(np.asarray(x).dtype)


# revision 21
# speedup vs baseline: 1.3116x; 1.0340x over previous
"""Trainium2 Bass kernel for nn_Attention2d (sparse_attention) — v3.

Math (per reference):
  x: (2, 128, 64, 64); T = 4096 tokens; 4 heads x 32 channels.
  qkv 1x1-conv -> per-head attention over T -> 1x1-conv out proj -> residual.

Sharding: one (batch, head) pair per core (8 cores). Each core computes its
head's attention fully on-chip and returns the per-head partial of the
output projection (bf16); the host sums the 4 head partials per batch and
adds the residual + biases (exact, fp32).

Structure (v3):
  - Scores are produced in supers of 3 s-chunks (1536 fp32 cols = 3 PSUM
    banks), with the st tile DOUBLE-buffered (6 banks): the PE runs QK for
    super n+1 while ScalarE/VectorE exp super n — no serial QK<->exp chain.
  - exp is SPLIT: ScalarE does activation-Exp on cols [0:ES); VectorE does
    a Schraudolph exp2 bit-trick (one tensor_scalar mult+add writing int16
    bit patterns that ARE the bf16 of 2^(y/128)) on the rest. Both carry
    the same global 2^((C2-16256)/128) factor, which cancels in softmax.
  - PV: one matmul per chunk into a [64,512] fp32 accumulator (32 v rows +
    32 ones rows giving the denominator), 32 accumulations per t-block.
  - epilogue per t-block: ScalarE copies the accumulator out of PSUM,
    GpSimd copies the denominator rows, VectorE fast-reciprocal +
    normalize-mul, PE out-projection, ScalarE output copy, bf16 DMA out
    (host accumulates partials in fp32).
  - all inputs arrive in 3 packed DMAs (x / weights / biases).
"""

import numpy as np
import ml_dtypes

B, C, Hh, Ww = 2, 128, 64, 64
T = Hh * Ww          # 4096
NH, CH = 4, 32
SCALE2 = float(1.0 / np.sqrt(CH))
N_CORES = 8
NCH = 32             # s-chunks (of 128 tokens) per t-block

# Schraudolph exp2 constants (VectorE path): bits = round(raw*C1 + C2),
# bits viewed as bf16 == 2^((bits-16256)/128) ~= exp(raw*SCALE2) * 2^((C2-16256)/128)
C1 = SCALE2 * float(np.log2(np.e)) * 128.0
C2 = 16256.0 - 7.4
# ScalarE path matches the global factor so both halves share one scale:
SBIAS = float((C2 - 16256.0) / 128.0 * np.log(2.0))

# super layout within a t-block: 10x 3-chunk + 1x 2-chunk
SUPERS = [(c, min(3, NCH - c)) for c in range(0, NCH, 3)]
ES3, ES2 = 896, 592  # ScalarE exp cols for 1536/1024-col supers

_cache = {}


def _build_nc(debug=False, zero_bias=False):
    import concourse.tile as tile
    from concourse import bacc, mybir

    BF16 = mybir.dt.bfloat16
    F32 = mybir.dt.float32
    I16 = mybir.dt.int16
    Exp = mybir.ActivationFunctionType.Exp
    MULT = mybir.AluOpType.mult
    ADD = mybir.AluOpType.add

    nc = bacc.Bacc("TRN2", target_bir_lowering=False, debug=False,
                   num_devices=N_CORES)
    dbg = {}
    if debug:
        dbg["q"] = nc.dram_tensor("dq", [128, T], BF16, kind="ExternalOutput")
        dbg["k"] = nc.dram_tensor("dk", [128, T], BF16, kind="ExternalOutput")
        dbg["vT"] = nc.dram_tensor("dvT", [128, 2048], BF16,
                                   kind="ExternalOutput")
        dbg["pv"] = nc.dram_tensor("dpv", [64, 512], F32,
                                   kind="ExternalOutput")
        dbg["rc"] = nc.dram_tensor("drc", [32, 512], F32,
                                   kind="ExternalOutput")
        dbg["an"] = nc.dram_tensor("dan", [32, T], BF16,
                                   kind="ExternalOutput")

    x_in = nc.dram_tensor("x", [128, T], BF16, kind="ExternalInput")
    # packed weights: wqT | wkT | wvT | wpT(padded to 128 rows)
    w_in = nc.dram_tensor("wpack", [128, 416], BF16, kind="ExternalInput")
    b_in = nc.dram_tensor("bpack", [128, 2], F32, kind="ExternalInput")
    out_t = nc.dram_tensor("out", [128, T], BF16, kind="ExternalOutput")

    with tile.TileContext(nc) as tc:
        with (
            tc.tile_pool(name="const", bufs=1) as cpool,
            tc.tile_pool(name="work", bufs=2) as wpool,
            tc.tile_pool(name="psum", bufs=1, space="PSUM") as pspool,
        ):
            x_sb = cpool.tile([128, T], BF16)
            nc.sync.dma_start(x_sb[:], x_in[:])
            w_sb = cpool.tile([128, 416], BF16)
            nc.sync.dma_start(w_sb[:], w_in[:])
            b_sb = cpool.tile([128, 2], F32)
            nc.sync.dma_start(b_sb[:], b_in[:])
            wq_sb = w_sb[:, 0:128]
            wk_sb = w_sb[:, 128:256]
            wv_sb = w_sb[:, 256:288]
            wp_sb = w_sb[0:32, 288:416]
            bq_sb = b_sb[:, 0:1]
            bk_sb = b_sb[:, 1:2]

            q_sb = cpool.tile([128, T], BF16)
            k_sb = cpool.tile([128, T], BF16)
            vT_sb = cpool.tile([128, 64 * NCH], BF16)  # (128, 2048)

            nc.gpsimd.memset(vT_sb[:], 1.0)
            sbias_sb = cpool.tile([128, 1], F32)
            nc.gpsimd.memset(sbias_sb[:], SBIAS)

            # 1-bank fp32 scratch for projections / vT / out-proj
            def scratch(nm):
                return pspool.tile([128, 512], F32, tag="pp", bufs=1, name=nm)

            def emit_proj(wsb, bsb, dst, c, nm, eng=None):
                ps = scratch(nm)
                nc.tensor.matmul(ps[:], wsb,
                                 x_sb[:, c * 512:(c + 1) * 512],
                                 start=True, stop=True)
                if zero_bias and eng == "s":
                    nc.scalar.copy(dst[:, c * 512:(c + 1) * 512], ps[:])
                else:
                    nc.vector.tensor_scalar_add(
                        dst[:, c * 512:(c + 1) * 512], ps[:], bsb)

            def emit_vt(half):
                ps = scratch(f"pp_v{half}")
                for j16 in range(16):
                    j = half * 16 + j16
                    nc.tensor.matmul(
                        ps[:, j16 * 32:(j16 + 1) * 32],
                        x_sb[:, j * 128:(j + 1) * 128],
                        wv_sb,
                        start=True, stop=True)
                src = ps[:].rearrange("p (j c) -> p j c", c=32)
                dstv = vT_sb[:].rearrange("p (j c) -> p j c", c=64)
                nc.vector.tensor_copy(
                    dstv[:, half * 16:(half + 1) * 16, 0:32], src)

            # ---- prologue: only what super 0 needs ----
            emit_proj(wk_sb, bk_sb, k_sb, 0, "pp_k0")
            emit_proj(wq_sb, bq_sb, q_sb, 0, "pp_q0")

            # ---- attention, software-pipelined over supers ----
            state = {}
            pv_tiles = {}
            supers_all = [(tb, c0, nch) for tb in range(8)
                          for (c0, nch) in SUPERS]

            def emit_score_exp(idx):
                tb, c0, nch = supers_all[idx]
                ncols = nch * 512
                es = ES3 if nch == 3 else ES2
                st = pspool.tile([128, 1536], F32, tag="st", bufs=2,
                                 name=f"st_{idx}")
                tsl = slice(tb * 512, (tb + 1) * 512)
                for ci in range(nch):
                    ch = c0 + ci
                    sblk = tb * 0 + ch  # s-chunk index (global over s)
                    nc.tensor.matmul(
                        st[:, ci * 512:(ci + 1) * 512],
                        k_sb[32 * (ch % 4):32 * (ch % 4) + 32,
                             128 * ch:128 * (ch + 1)],
                        q_sb[32 * (ch % 4):32 * (ch % 4) + 32, tsl],
                        start=True, stop=True,
                        tile_position=(32 * (ch % 4), 0))
                p_sb = wpool.tile([128, 1536], BF16, tag="p", bufs=3)
                nc.scalar.activation(p_sb[:, 0:es], st[:, 0:es], Exp,
                                     bias=sbias_sb[:], scale=SCALE2)
                nc.vector.tensor_scalar(
                    p_sb[:, es:ncols].bitcast(I16), st[:, es:ncols],
                    C1, C2, MULT, ADD)
                state[idx] = p_sb

            def emit_pv(idx):
                tb, c0, nch = supers_all[idx]
                p_sb = state.pop(idx)
                if c0 == 0:
                    pv_tiles[tb] = pspool.tile(
                        [64, 512], F32, tag="pv", bufs=1, name=f"pv_{tb}")
                pv = pv_tiles[tb]
                for ci in range(nch):
                    ch = c0 + ci
                    nc.tensor.matmul(
                        pv[:],
                        vT_sb[:, 64 * ch:64 * (ch + 1)],
                        p_sb[:, ci * 512:(ci + 1) * 512],
                        start=(ch == 0), stop=(ch == NCH - 1),
                        skip_group_check=True)
                if c0 + nch == NCH:
                    # t-block epilogue: numerator/denominator copied to
                    # partition-base 0 by ScalarE (proven shift-capable),
                    # then unshifted reciprocal (VectorE) and normalize
                    # multiply (GpSimd, fresh un-offset output tile)
                    a_num = wpool.tile([32, 512], F32, tag="anum")
                    nc.scalar.copy(a_num[:], pv[0:32, :])
                    a_den = wpool.tile([32, 512], F32, tag="aden")
                    nc.scalar.copy(a_den[:], pv[32:64, :])
                    rc = wpool.tile([32, 512], F32, tag="rc")
                    nc.vector.reciprocal_approx_fast(rc[:], a_den[:])
                    an_t = wpool.tile([32, 512], BF16, tag="an")
                    nc.gpsimd.tensor_mul(an_t[:], a_num[:], rc[:])
                    if debug and tb == 0:
                        dpv_sb = wpool.tile([64, 512], F32, tag="dbgpv")
                        nc.vector.tensor_copy(dpv_sb[:], pv[:])
                        nc.sync.dma_start(dbg["pv"][:], dpv_sb[:])
                        nc.sync.dma_start(dbg["rc"][:], rc[:])
                        nc.sync.dma_start(dbg["an"][:, 0:512], an_t[:])
                    op = scratch(f"pp_o{tb}")
                    nc.tensor.matmul(op[:], wp_sb, an_t[:],
                                     start=True, stop=True)
                    o_sb = wpool.tile([128, 512], BF16, tag="o")
                    nc.scalar.copy(o_sb[:], op[:])
                    nc.sync.dma_start(
                        out_t[:, tb * 512:(tb + 1) * 512], o_sb[:])

            for idx in range(len(supers_all)):
                emit_score_exp(idx)
                # stagger input prep into the supers that have slack,
                # ahead of the super that needs it
                if idx == 0:
                    emit_vt(0)
                if idx == 1:
                    emit_vt(1)
                if 0 <= idx <= 6:
                    emit_proj(wk_sb, bk_sb, k_sb, idx + 1, f"pp_k{idx + 1}",
                              eng="s" if idx % 2 == 0 else None)
                if idx % 11 == 5 and idx < 77:
                    c = idx // 11 + 1
                    emit_proj(wq_sb, bq_sb, q_sb, c, f"pp_q{c}",
                              eng="s" if c % 2 == 0 else None)
                if idx >= 1:
                    emit_pv(idx - 1)
            emit_pv(len(supers_all) - 1)
            if debug:
                nc.sync.dma_start(dbg["q"][:], q_sb[:])
                nc.sync.dma_start(dbg["k"][:], k_sb[:])
                nc.sync.dma_start(dbg["vT"][:], vT_sb[:])

    nc.compile()
    return nc


def _get_nc(debug=False, zero_bias=False):
    key = ("nc", debug, zero_bias)
    if key not in _cache:
        _cache[key] = _build_nc(debug, zero_bias)
    return _cache[key]


def _make_in_maps(x_, w_qkv, b_qkv, w_proj):
    bf16 = ml_dtypes.bfloat16
    in_maps = []
    for core in range(N_CORES):
        b, g = divmod(core, NH)
        wq = w_qkv[96 * g:96 * g + 32]
        wk = w_qkv[96 * g + 32:96 * g + 64]
        wv = w_qkv[96 * g + 64:96 * g + 96]
        wpack = np.zeros((128, 416), np.float32)
        wpack[:, 0:128] = np.tile(wq, (4, 1)).T
        wpack[:, 128:256] = np.tile(wk, (4, 1)).T
        wpack[:, 256:288] = wv.T
        wpack[0:32, 288:416] = w_proj[:, 32 * g:32 * (g + 1)].T
        bpack = np.stack([np.tile(b_qkv[96 * g:96 * g + 32], 4),
                          np.tile(b_qkv[96 * g + 32:96 * g + 64], 4)],
                         axis=1)
        in_maps.append({
            "x": x_[b].astype(bf16),
            "wpack": np.ascontiguousarray(wpack).astype(bf16),
            "bpack": np.ascontiguousarray(bpack.astype(np.float32)),
        })
    return in_maps


def _run(x, w_qkv, b_qkv, w_proj, b_proj, trace=False):
    from concourse.bass_utils import run_bass_kernel_spmd

    x_ = np.ascontiguousarray(np.asarray(x, np.float32).reshape(B, C, T))
    w_qkv = np.asarray(w_qkv, np.float32)
    b_qkv = np.asarray(b_qkv, np.float32)
    w_proj = np.asarray(w_proj, np.float32)
    b_proj = np.asarray(b_proj, np.float32)
    nc = _get_nc(zero_bias=not np.any(b_qkv))

    in_maps = _make_in_maps(x_, w_qkv, b_qkv, w_proj)
    res = run_bass_kernel_spmd(nc, in_maps, core_ids=list(range(N_CORES)),
                               trace=trace)
    out = np.empty((B, C, T), np.float32)
    for b in range(B):
        acc = x_[b] + b_proj[:, None]
        for g in range(NH):
            wp = w_proj[:, 32 * g:32 * (g + 1)]
            bv = b_qkv[96 * g + 64:96 * g + 96]
            acc = acc + res.results[NH * b + g]["out"].astype(np.float32) \
                + (wp @ bv)[:, None]
        out[b] = acc
    return out.reshape(B, C, Hh, Ww), res


def kernel(x, w_qkv, b_qkv, w_proj, b_proj):
    out, _ = _run(x, w_qkv, b_qkv, w_proj, b_proj, trace=False)
    return out.astype(np.asarray(x).dtype)
